# revision 18
# baseline (speedup 1.0000x reference)
"""Trainium2 Bass kernel: single-head attention block (B=4, S=2048, E=1024).

Reference computation (per batch b):
    Q = x@W1+b1; K = x@W2+b2; V = x@W3+b3
    out = softmax(Q K^T / 32) V @ W4 + b4

Algebraic restructuring (host folds weights, softmax invariances):
    scores_ij = x_i W1 W2^T x_j^T / 32^2-ish + (x W1 b2)_i + (b1 W2^T x^T)_j + b1 b2
  Softmax over j kills any term constant in j, so with W12 := W1 W2^T and
  ktil_j := x_j (W2 b1) + b1.b2 the probabilities need only ONE projection
  M = x W12 instead of Q and K.  Likewise P V W4 = P (x W34) + b3 W4 with
  W34 := W3 W4, so V and the output projection collapse into VW = x W34 and
  the attention-weighted sum IS the final output (up to host-applied
  normalization and the folded bias b4' = b3 W4 + b4).  Device matmuls:
    MT  = (XH^T W12s)^T   [E, SQ]   (bf16, W12s = 32*W12)
    VW  = XH^T W34s       [SQ, E]   (bf16 -> fp8, AllGather halves)
    S^T = XF^T-blocks . MT          (fp8 DoubleRow)   -> exp -> PX' = PX - mu
    sums = 1^T PX'                  (fp8 DoubleRow)
    OT  = VW^T-blocks . PX'         (fp8 DoubleRow)   -> bf16 -> DRAM
  Host: out[i,f] = (OT^T + mu*colsum(x W34s)) / (32*(sums_i + mu*S)) + b4'.
  Centering PX by mu ~= E[exp(s)] plus the exact host colsum keeps the fp8
  quantization error of PX/VW to ~9e-3 end-to-end (vs 1.9e-2 naive fp8).

Sharding: 8 cores = (batch b, seq-half h); each core owns 1024 query rows.
Scores need no collective (full x^T is an input, fed fp8); only the 1 MB
fp8 VW halves are exchanged pairwise, overlapped with the scores phase.

Simulated end-to-end l2 relative error vs fp32 reference: ~9.0e-3.
"""

from contextlib import ExitStack

import ml_dtypes
import numpy as np

import concourse.tile as tile
from concourse import bacc, mybir
from concourse.bass_utils import run_bass_kernel_spmd

BF16 = mybir.dt.bfloat16
F8 = mybir.dt.float8e4
F32 = mybir.dt.float32
AF = mybir.ActivationFunctionType
DR = mybir.MatmulPerfMode.DoubleRow
NP_BF16 = ml_dtypes.bfloat16
NP_F8 = ml_dtypes.float8_e4m3

B, S, E = 4, 2048, 1024
SQ = S // 2          # query rows per core
NCORES = 8
P = 128              # partitions
NB = 512             # matmul moving free-dim (one fp32 PSUM bank)
PAIRS = [[0, 1], [2, 3], [4, 5], [6, 7]]
SW = 32.0            # host scale on W12/W34 (keeps fp8 operands in range)
MU = float(np.exp(1 / 18.0))   # ~E[exp(score)] for this input distribution
ET, ST, QT = E // P, S // P, SQ // P
QC = SQ // NB        # query 512-chunks per core (2)
ED, SD = ET // 2, ST // 2      # DoubleRow pair-tiles over E / S


FP8 = True          # DoubleRow fp8 for scores / sums / OT
VW_FP8 = True       # fp8 VW projection (error mean-corrected via host colsum)
MT_FP8 = False      # fp8 MT projection (adds ~3e-3 error, saves ~8us)
SUMS_OFFLOAD = True  # softmax denominators on DVE+GpSimd instead of the PE


def emit_attention(tc, aps, fp8=FP8, vw_fp8=VW_FP8, mt_fp8=MT_FP8,
                   sums_offload=SUMS_OFFLOAD, ps1_bufs=6, sc_bufs=7,
                   warmup=0):
    nc = tc.nc
    xh_d, xh8_d, xf_d, w12_d, w34_d, ktb_d, out_d, sums_d = aps
    XDT = F8 if fp8 else BF16
    vw_fp8 = vw_fp8 and fp8
    mt_fp8 = mt_fp8 and fp8

    def r128(ap):  # [(t p), n] -> [t, p, n]
        return ap.rearrange("(t p) n -> t p n", p=P)

    cnt = [0]

    def copy_ps(dst, ps):
        """PSUM->SBUF copy alternating DVE/ACT to balance engine load."""
        if cnt[0] % 2 == 0:
            nc.vector.tensor_copy(dst, ps)
        else:
            nc.scalar.copy(dst, ps)
        cnt[0] += 1

    with ExitStack() as ctx:
        persist = ctx.enter_context(tc.tile_pool(name="persist", bufs=1))
        dram = ctx.enter_context(tc.tile_pool(name="dram", bufs=1, space="DRAM"))
        xf_s = persist.tile([P, ET, S], XDT, tag="xf")
        mt = persist.tile([P, ET, SQ], XDT, tag="mt")
        vw = persist.tile([P, ST, E], XDT, tag="vw")
        px = persist.tile([P, ST, SQ], XDT, tag="px")
        ktb_s = persist.tile([P, ST], F32, tag="ktb")
        sums_sb = persist.tile([1, SQ], F32, tag="sums_sb")
        vwloc = dram.tile([SQ, E], XDT, tag="vwloc")
        vwglob = dram.tile([2, SQ, E], XDT, tag="vwglob")
        if not sums_offload:
            # pair-dim step must be 16B-aligned for DoubleRow ldweights
            ones = persist.tile([P, 2, 16], XDT, tag="ones")
            nc.gpsimd.memset(ones[:], 1.0)
        nc.sync.dma_start(ktb_s[:], ktb_d)

        # ---- Phase 1: MT projection, VW projection (own half) + gather ----
        with (
            tc.tile_pool(name="p1", bufs=1) as p1,
            tc.tile_pool(name="ps1", bufs=ps1_bufs, space="PSUM") as ps1,
        ):
            MDT = F8 if mt_fp8 else BF16
            VDT = F8 if vw_fp8 else BF16
            xh_s = p1.tile([P, ET, SQ], MDT, tag="xh")
            w12_s = p1.tile([P, ET, E], MDT, tag="w12")
            xv_s = p1.tile([P, ET, SQ], VDT, tag="xv")
            w34_s = p1.tile([P, ET, E], VDT, tag="w34")
            xh_src = xh8_d if mt_fp8 else xh_d
            xv_src = xh8_d if vw_fp8 else xh_d

            # PE warmup during the initial DMA: ~4us of throwaway matmuls
            # flips HAM to the 2.4 GHz clock before real work arrives.
            if warmup:
                scr = p1.tile([P, NB], BF16, tag="scr")
                nc.gpsimd.memset(scr[:], 0.0)
                psw = ps1.tile([P, NB], F32, name="ps", tag="ps")
                for i in range(warmup):
                    nc.tensor.matmul(psw[:], scr[:, 0:P], scr[:],
                                     start=(i == 0), stop=(i == warmup - 1))

            # DMA in consumption order: w12 in ft-column slices so the first
            # MT groups unblock after ~a quarter of the weight transfer.
            nc.sync.dma_start(xh_s[:, 0], r128(xh_src)[0])
            for e in range(ET):
                nc.sync.dma_start(w12_s[:, e, 0:2 * P], r128(w12_d)[e][:, 0:2 * P])
            for t in range(1, ET):
                nc.sync.dma_start(xh_s[:, t], r128(xh_src)[t])
            for fp in range(1, ET // 2):
                for e in range(ET):
                    nc.sync.dma_start(
                        w12_s[:, e, fp * 2 * P:(fp + 1) * 2 * P],
                        r128(w12_d)[e][:, fp * 2 * P:(fp + 1) * 2 * P])
            for t in range(ET):
                nc.sync.dma_start(xv_s[:, t], r128(xv_src)[t])
                nc.sync.dma_start(w34_s[:, t], r128(w34_d)[t])
            for t in range(ET):
                nc.sync.dma_start(xf_s[:, t], r128(xf_d)[t])

            # MT[f, i] = (XH^T W12s)^T: stationary w12-block, both q-chunks.
            for ft in range(ET):
                pss = [ps1.tile([P, NB], F32, name="ps", tag="ps")
                       for _ in range(QC)]
                if mt_fp8:
                    for ed in range(ED):
                        lhsT = w12_s[:, 2 * ed:2 * ed + 2, ft * P:(ft + 1) * P]
                        for c in range(QC):
                            nc.tensor.matmul(
                                pss[c][:], lhsT,
                                xh_s[:, 2 * ed:2 * ed + 2, c * NB:(c + 1) * NB],
                                start=(ed == 0), stop=(ed == ED - 1),
                                perf_mode=DR)
                else:
                    for e in range(ET):
                        for c in range(QC):
                            nc.tensor.matmul(
                                pss[c][:], w12_s[:, e, ft * P:(ft + 1) * P],
                                xh_s[:, e, c * NB:(c + 1) * NB],
                                start=(e == 0), stop=(e == ET - 1))
                for c in range(QC):
                    copy_ps(mt[:, ft, c * NB:(c + 1) * NB], pss[c][:])

            # VW-own[j, f] = XH^T W34s into local tile slots 0..QT-1, then
            # pairwise AllGather; loadback fills the global [ST, E] layout.
            for st in range(QT):
                pss = [ps1.tile([P, NB], F32, name="ps", tag="ps")
                       for _ in range(E // NB)]
                if vw_fp8:
                    for ed in range(ED):
                        lhsT = xv_s[:, 2 * ed:2 * ed + 2, st * P:(st + 1) * P]
                        for c in range(E // NB):
                            nc.tensor.matmul(
                                pss[c][:], lhsT,
                                w34_s[:, 2 * ed:2 * ed + 2, c * NB:(c + 1) * NB],
                                start=(ed == 0), stop=(ed == ED - 1),
                                perf_mode=DR)
                else:
                    for e in range(ET):
                        for c in range(E // NB):
                            nc.tensor.matmul(
                                pss[c][:], xv_s[:, e, st * P:(st + 1) * P],
                                w34_s[:, e, c * NB:(c + 1) * NB],
                                start=(e == 0), stop=(e == ET - 1))
                for c in range(E // NB):
                    copy_ps(vw[:, st, c * NB:(c + 1) * NB], pss[c][:])
                nc.sync.dma_start(r128(vwloc[:])[st], vw[:, st, :])
            nc.gpsimd.collective_compute(
                "AllGather", mybir.AluOpType.bypass, replica_groups=PAIRS,
                ins=[vwloc.opt()], outs=[vwglob.opt()],
            )
            for hh in range(2):
                vg = r128(vwglob[hh])
                for st in range(QT):
                    nc.sync.dma_start(vw[:, hh * QT + st, :], vg[st])

        # ---- Phases 2-4: scores+exp, sums, attention-weighted output ----
        with (
            tc.tile_pool(name="p2c", bufs=4) as p2c,
            tc.tile_pool(name="ps_sc", bufs=sc_bufs, space="PSUM") as ps_sc,
            tc.tile_pool(name="ps_tp", bufs=1, space="PSUM") as ps_tp,
        ):
            # Scores^T tiles [j, i] via DoubleRow: lhsT = XF pair-block,
            # rhs = MT pair-rows; exp(s) - mu lands in px (fp8).
            for jt in range(ST):
                pss = [ps_sc.tile([P, NB], F32, name="sc", tag="sc")
                       for _ in range(QC)]
                for ed in range(ED):
                    lhsT = xf_s[:, 2 * ed:2 * ed + 2, jt * P:(jt + 1) * P]
                    for c in range(QC):
                        if fp8:
                            nc.tensor.matmul(
                                pss[c][:], lhsT,
                                mt[:, 2 * ed:2 * ed + 2, c * NB:(c + 1) * NB],
                                start=(ed == 0), stop=(ed == ED - 1),
                                perf_mode=DR)
                        else:
                            for k in range(2):
                                nc.tensor.matmul(
                                    pss[c][:],
                                    xf_s[:, 2 * ed + k, jt * P:(jt + 1) * P],
                                    mt[:, 2 * ed + k, c * NB:(c + 1) * NB],
                                    start=(ed == 0 and k == 0),
                                    stop=(ed == ED - 1 and k == 1))
                for c in range(QC):
                    pxb = p2c.tile([P, NB], BF16, name="pxb", tag="pxb")
                    nc.scalar.activation(pxb[:], pss[c][:], AF.Exp,
                                         scale=1.0 / (SW * 32.0),
                                         bias=ktb_s[:, jt:jt + 1])
                    nc.vector.tensor_scalar_sub(
                        px[:, jt, c * NB:(c + 1) * NB], pxb[:], MU)

            # sums[i] = 1^T PX' (partition reduce).
            if sums_offload:
                # DVE accumulation chain over the 16 j-tiles, then a GpSimd
                # cross-partition all-reduce: zero TensorE cycles.
                ADD, MUL = mybir.AluOpType.add, mybir.AluOpType.mult
                import concourse.bass_isa as bass_isa
                for c in range(QC):
                    acc = p2c.tile([P, NB], F32, name="sacc", tag="sacc")
                    red = p2c.tile([P, NB], F32, name="sred", tag="sred")
                    nc.vector.scalar_tensor_tensor(
                        acc[:], px[:, 0, c * NB:(c + 1) * NB], 1.0,
                        px[:, 1, c * NB:(c + 1) * NB], MUL, ADD)
                    for T in range(2, ST):
                        nc.vector.scalar_tensor_tensor(
                            acc[:], acc[:], 1.0,
                            px[:, T, c * NB:(c + 1) * NB], MUL, ADD)
                    nc.gpsimd.partition_all_reduce(
                        red[:], acc[:], channels=P,
                        reduce_op=bass_isa.ReduceOp.add)
                    nc.vector.tensor_copy(sums_sb[:, c * NB:(c + 1) * NB],
                                          red[0:1, :])
            else:
                for c in range(QC):
                    ps = ps_tp.tile([1, NB], F32, name="pssum", tag="pssum")
                    for T in range(SD):
                        if fp8:
                            nc.tensor.matmul(
                                ps[:], ones[:, :, 0:1],
                                px[:, 2 * T:2 * T + 2, c * NB:(c + 1) * NB],
                                start=(T == 0), stop=(T == SD - 1),
                                perf_mode=DR)
                        else:
                            for k in range(2):
                                nc.tensor.matmul(
                                    ps[:], ones[:, k, 0:1],
                                    px[:, 2 * T + k, c * NB:(c + 1) * NB],
                                    start=(T == 0 and k == 0),
                                    stop=(T == SD - 1 and k == 1))
                    nc.vector.tensor_copy(sums_sb[:, c * NB:(c + 1) * NB],
                                          ps[:])
            nc.sync.dma_start(sums_d, sums_sb[:])

            # OT[f, i] = VW^T PX' -> bf16 -> DRAM (normalization on host).
            for ft in range(ET):
                pss = [ps_sc.tile([P, NB], F32, name="sc", tag="sc")
                       for _ in range(QC)]
                for T in range(SD):
                    lhsT = vw[:, 2 * T:2 * T + 2, ft * P:(ft + 1) * P]
                    for c in range(QC):
                        if fp8:
                            nc.tensor.matmul(
                                pss[c][:], lhsT,
                                px[:, 2 * T:2 * T + 2, c * NB:(c + 1) * NB],
                                start=(T == 0), stop=(T == SD - 1),
                                perf_mode=DR)
                        else:
                            for k in range(2):
                                nc.tensor.matmul(
                                    pss[c][:],
                                    vw[:, 2 * T + k, ft * P:(ft + 1) * P],
                                    px[:, 2 * T + k, c * NB:(c + 1) * NB],
                                    start=(T == 0 and k == 0),
                                    stop=(T == SD - 1 and k == 1))
                for c in range(QC):
                    ot_t = p2c.tile([P, NB], BF16, name="ot", tag="ot")
                    copy_ps(ot_t[:], pss[c][:])
                    nc.sync.dma_start(
                        out_d[ft * P:(ft + 1) * P, c * NB:(c + 1) * NB],
                        ot_t[:])


def build_program(num_devices=NCORES, repeats=1, **emit_kw):
    nc = bacc.Bacc("TRN2", target_bir_lowering=False, debug=False,
                   num_devices=num_devices)
    fp8 = emit_kw.get("fp8", FP8)
    vw_fp8 = emit_kw.get("vw_fp8", VW_FP8) and fp8
    mt_fp8 = emit_kw.get("mt_fp8", MT_FP8) and fp8
    XDT = F8 if fp8 else BF16
    aps = (
        nc.dram_tensor("xh", [E, SQ], BF16, kind="ExternalInput").ap(),
        nc.dram_tensor("xh8", [E, SQ], F8 if fp8 else BF16,
                       kind="ExternalInput").ap(),
        nc.dram_tensor("xf", [E, S], XDT, kind="ExternalInput").ap(),
        nc.dram_tensor("w12", [E, E], F8 if mt_fp8 else BF16,
                       kind="ExternalInput").ap(),
        nc.dram_tensor("w34", [E, E], F8 if vw_fp8 else BF16,
                       kind="ExternalInput").ap(),
        nc.dram_tensor("ktb", [P, ST], F32, kind="ExternalInput").ap(),
        nc.dram_tensor("out", [E, SQ], BF16, kind="ExternalOutput").ap(),
        nc.dram_tensor("sums", [1, SQ], F32, kind="ExternalOutput").ap(),
    )
    with tile.TileContext(nc) as tc:
        for _ in range(repeats):
            emit_attention(tc, aps, **emit_kw)
    nc.compile()
    return nc


def host_prep(x, W1, b1, W2, b2, W3, b3, W4, b4, fp8=None, vw_fp8=None,
              mt_fp8=None):
    if fp8 is None: fp8 = FP8
    if vw_fp8 is None: vw_fp8 = VW_FP8
    if mt_fp8 is None: mt_fp8 = MT_FP8
    """Fold weights / biases; build per-core input dicts + postproc consts."""
    f32 = np.float32
    vw_fp8 = vw_fp8 and fp8
    mt_fp8 = mt_fp8 and fp8
    W12 = (W1.astype(f32) @ W2.T.astype(f32)) * f32(SW)
    W34 = (W3.astype(f32) @ W4.astype(f32)) * f32(SW)
    b4p = (b3.astype(np.float64) @ W4.astype(np.float64) + b4).astype(f32)
    w21 = (W2.astype(f32) @ b1.astype(f32))
    c21 = float(b2.astype(np.float64) @ b1.astype(np.float64))
    np_x = NP_F8 if fp8 else NP_BF16
    ws = {"w12": np.ascontiguousarray(W12.astype(NP_F8 if mt_fp8 else NP_BF16)),
          "w34": np.ascontiguousarray(W34.astype(NP_F8 if vw_fp8 else NP_BF16))}
    in_maps, css = [], []
    for b in range(B):
        xb = np.asarray(x[b], f32)
        xT = np.ascontiguousarray(xb.T)
        xf = np.ascontiguousarray(xT.astype(np_x))
        ktil = (xb @ w21 + c21) / f32(32.0)
        ktb = np.ascontiguousarray(ktil.reshape(ST, P).T.astype(f32))
        css.append((xb.sum(0) @ W34).astype(f32))
        for h in range(2):
            xh = np.ascontiguousarray(xT[:, h * SQ:(h + 1) * SQ].astype(NP_BF16))
            xh8 = np.ascontiguousarray(xf[:, h * SQ:(h + 1) * SQ])
            in_maps.append({"xh": xh, "xh8": xh8, "xf": xf, "ktb": ktb, **ws})
    return in_maps, css, b4p


def make_in_maps(x, W1, b1, W2, b2, W3, b3, W4, b4):
    return host_prep(x, W1, b1, W2, b2, W3, b3, W4, b4)[0]


_PROGRAM = None


def kernel(x, W1, b1, W2, b2, W3, b3, W4, b4):
    x, W1, b1, W2, b2, W3, b3, W4, b4 = (
        np.asarray(a) for a in (x, W1, b1, W2, b2, W3, b3, W4, b4))
    global _PROGRAM
    if _PROGRAM is None:
        _PROGRAM = build_program()
    nc = _PROGRAM
    in_maps, css, b4p = host_prep(x, W1, b1, W2, b2, W3, b3, W4, b4)
    res = run_bass_kernel_spmd(nc, in_maps, core_ids=list(range(NCORES)))
    out = np.empty((B, S, E), np.float32)
    for i in range(NCORES):
        b, h = divmod(i, 2)
        ot = np.asarray(res.results[i]["out"]).astype(np.float32)  # [E, SQ]
        sums = np.asarray(res.results[i]["sums"])[0].astype(np.float32)
        dst = out[b, h * SQ:(h + 1) * SQ, :]
        np.multiply(ot.T + MU * css[b][None, :],
                    (1.0 / (SW * (sums + MU * S)))[:, None], out=dst)
        dst += b4p[None, :]
    return out


# revision 19
# speedup vs baseline: 1.1720x; 1.1720x over previous
"""Trainium2 Bass kernel: single-head attention block (B=4, S=2048, E=1024).

Reference computation (per batch b):
    Q = x@W1+b1; K = x@W2+b2; V = x@W3+b3
    out = softmax(Q K^T / 32) V @ W4 + b4

Algebraic restructuring (host folds weights, softmax invariances):
    scores_ij = x_i W1 W2^T x_j^T / 32^2-ish + (x W1 b2)_i + (b1 W2^T x^T)_j + b1 b2
  Softmax over j kills any term constant in j, so with W12 := W1 W2^T and
  ktil_j := x_j (W2 b1) + b1.b2 the probabilities need only ONE projection
  M = x W12 instead of Q and K.  Likewise P V W4 = P (x W34) + b3 W4 with
  W34 := W3 W4, so V and the output projection collapse into VW = x W34 and
  the attention-weighted sum IS the final output (up to host-applied
  normalization and the folded bias b4' = b3 W4 + b4).  Device matmuls:
    MT  = (XH^T W12s)^T   [E, SQ]   (bf16, W12s = 32*W12)
    VW  = XH^T W34s       [SQ, E]   (fp8 DoubleRow, AllGather halves)
    S^T = XF^T-blocks . MT          (fp8 DoubleRow)   -> exp -> PX' = PX - mu
    sums = colsum(PX')              (DVE chain + GpSimd partition reduce)
    OT  = VW^T-blocks . PX'         (fp8 DoubleRow)   -> bf16 -> DRAM
  Host: out[i,f] = (OT^T + mu*colsum(x W34s)) / (32*(sums_i + mu*S)) + b4'.
  Centering PX by mu ~= E[exp(s)] plus the exact host colsum keeps the fp8
  quantization error of PX/VW to ~9e-3 end-to-end (vs 1.9e-2 naive fp8).

Sharding: 8 cores = (batch b, seq-half h); each core owns 1024 query rows.
Scores need no collective (full x^T is an input, fed fp8); only the 1 MB
fp8 VW halves are exchanged pairwise, overlapped with the scores phase.

Measured end-to-end l2 relative error vs fp32 reference: ~1.16e-2.
"""

from contextlib import ExitStack

import ml_dtypes
import numpy as np

import concourse.tile as tile
from concourse import bacc, bass_isa, mybir
from concourse.bass_utils import run_bass_kernel_spmd

BF16 = mybir.dt.bfloat16
F8 = mybir.dt.float8e4
F32 = mybir.dt.float32
AF = mybir.ActivationFunctionType
DR = mybir.MatmulPerfMode.DoubleRow
NP_BF16 = ml_dtypes.bfloat16
NP_F8 = ml_dtypes.float8_e4m3

B, S, E = 4, 2048, 1024
SQ = S // 2          # query rows per core
NCORES = 8
P = 128              # partitions
NB = 512             # matmul moving free-dim (one fp32 PSUM bank)
PAIRS = [[0, 1], [2, 3], [4, 5], [6, 7]]
SW = 32.0            # host scale on W12/W34 (keeps fp8 operands in range)
MU = float(np.exp(1 / 18.0))   # ~E[exp(score)] for this input distribution
ET, ST, QT = E // P, S // P, SQ // P
QC = SQ // NB        # query 512-chunks per core (2)
ED, SD = ET // 2, ST // 2      # DoubleRow pair-tiles over E / S


FP8 = True          # DoubleRow fp8 for scores / sums / OT
VW_FP8 = True       # fp8 VW projection (error mean-corrected via host colsum)
MT_FP8 = False      # fp8 MT projection (adds ~3e-3 error, saves ~8us)
SUMS_OFFLOAD = True  # softmax denominators on DVE+GpSimd instead of the PE


def emit_attention(tc, aps, fp8=FP8, vw_fp8=VW_FP8, mt_fp8=MT_FP8,
                   sums_offload=SUMS_OFFLOAD, ps1_bufs=6, sc_bufs=7,
                   warmup=0):
    nc = tc.nc
    xh_d, xh8_d, xf_d, w12_d, w34_d, ktb_d, out_d, sums_d = aps
    XDT = F8 if fp8 else BF16
    vw_fp8 = vw_fp8 and fp8
    mt_fp8 = mt_fp8 and fp8

    def r128(ap):  # [(t p), n] -> [t, p, n]
        return ap.rearrange("(t p) n -> t p n", p=P)

    cnt = [0]

    def copy_ps(dst, ps):
        """PSUM->SBUF copy alternating DVE/ACT to balance engine load."""
        if cnt[0] % 2 == 0:
            nc.vector.tensor_copy(dst, ps)
        else:
            nc.scalar.copy(dst, ps)
        cnt[0] += 1

    with ExitStack() as ctx:
        persist = ctx.enter_context(tc.tile_pool(name="persist", bufs=1))
        dram = ctx.enter_context(tc.tile_pool(name="dram", bufs=1, space="DRAM"))
        xf_s = persist.tile([P, ET, S], XDT, tag="xf")
        mt = persist.tile([P, ET, SQ], XDT, tag="mt")
        vw = persist.tile([P, ST, E], XDT, tag="vw")
        px = persist.tile([P, ST, SQ], XDT, tag="px")
        ktb_s = persist.tile([P, ST], F32, tag="ktb")
        sums_sb = persist.tile([1, SQ], F32, tag="sums_sb")
        vwloc = dram.tile([SQ, E], XDT, tag="vwloc")
        vwglob = dram.tile([2, SQ, E], XDT, tag="vwglob")
        if not sums_offload:
            # pair-dim step must be 16B-aligned for DoubleRow ldweights
            ones = persist.tile([P, 2, 16], XDT, tag="ones")
            nc.gpsimd.memset(ones[:], 1.0)
        nc.sync.dma_start(ktb_s[:], ktb_d)

        # ---- Phase 1: MT projection, VW projection (own half) + gather ----
        with (
            tc.tile_pool(name="p1", bufs=1) as p1,
            tc.tile_pool(name="ps1", bufs=ps1_bufs, space="PSUM") as ps1,
        ):
            MDT = F8 if mt_fp8 else BF16
            VDT = F8 if vw_fp8 else BF16
            xh_s = p1.tile([P, ET, SQ], MDT, tag="xh")
            w12_s = p1.tile([P, ET, E], MDT, tag="w12")
            xv_s = p1.tile([P, ET, SQ], VDT, tag="xv")
            w34_s = p1.tile([P, ET, E], VDT, tag="w34")
            xh_src = xh8_d if mt_fp8 else xh_d
            xv_src = xh8_d if vw_fp8 else xh_d

            # PE warmup during the initial DMA: ~4us of throwaway matmuls
            # flips HAM to the 2.4 GHz clock before real work arrives.
            if warmup:
                scr = p1.tile([P, NB], BF16, tag="scr")
                nc.gpsimd.memset(scr[:], 0.0)
                psw = ps1.tile([P, NB], F32, name="ps", tag="ps")
                for i in range(warmup):
                    nc.tensor.matmul(psw[:], scr[:, 0:P], scr[:],
                                     start=(i == 0), stop=(i == warmup - 1))

            # DMA in consumption order: w12 in ft-column slices so the first
            # MT groups unblock after ~a quarter of the weight transfer.
            nc.sync.dma_start(xh_s[:, 0], r128(xh_src)[0])
            for e in range(ET):
                nc.sync.dma_start(w12_s[:, e, 0:2 * P], r128(w12_d)[e][:, 0:2 * P])
            for t in range(1, ET):
                nc.sync.dma_start(xh_s[:, t], r128(xh_src)[t])
            for fp in range(1, ET // 2):
                for e in range(ET):
                    nc.sync.dma_start(
                        w12_s[:, e, fp * 2 * P:(fp + 1) * 2 * P],
                        r128(w12_d)[e][:, fp * 2 * P:(fp + 1) * 2 * P])
            for t in range(ET):
                nc.sync.dma_start(xv_s[:, t], r128(xv_src)[t])
                nc.sync.dma_start(w34_s[:, t], r128(w34_d)[t])
            for t in range(ET):
                nc.sync.dma_start(xf_s[:, t], r128(xf_d)[t])

            # MT[f, i] = (XH^T W12s)^T: stationary w12-block, both q-chunks.
            for ft in range(ET):
                pss = [ps1.tile([P, NB], F32, name="ps", tag="ps")
                       for _ in range(QC)]
                if mt_fp8:
                    for ed in range(ED):
                        lhsT = w12_s[:, 2 * ed:2 * ed + 2, ft * P:(ft + 1) * P]
                        for c in range(QC):
                            nc.tensor.matmul(
                                pss[c][:], lhsT,
                                xh_s[:, 2 * ed:2 * ed + 2, c * NB:(c + 1) * NB],
                                start=(ed == 0), stop=(ed == ED - 1),
                                perf_mode=DR)
                else:
                    for e in range(ET):
                        for c in range(QC):
                            nc.tensor.matmul(
                                pss[c][:], w12_s[:, e, ft * P:(ft + 1) * P],
                                xh_s[:, e, c * NB:(c + 1) * NB],
                                start=(e == 0), stop=(e == ET - 1))
                for c in range(QC):
                    copy_ps(mt[:, ft, c * NB:(c + 1) * NB], pss[c][:])

            # VW-own[j, f] = XH^T W34s into local tile slots 0..QT-1, then
            # pairwise AllGather; loadback fills the global [ST, E] layout.
            for st in range(QT):
                pss = [ps1.tile([P, NB], F32, name="ps", tag="ps")
                       for _ in range(E // NB)]
                if vw_fp8:
                    for ed in range(ED):
                        lhsT = xv_s[:, 2 * ed:2 * ed + 2, st * P:(st + 1) * P]
                        for c in range(E // NB):
                            nc.tensor.matmul(
                                pss[c][:], lhsT,
                                w34_s[:, 2 * ed:2 * ed + 2, c * NB:(c + 1) * NB],
                                start=(ed == 0), stop=(ed == ED - 1),
                                perf_mode=DR)
                else:
                    for e in range(ET):
                        for c in range(E // NB):
                            nc.tensor.matmul(
                                pss[c][:], xv_s[:, e, st * P:(st + 1) * P],
                                w34_s[:, e, c * NB:(c + 1) * NB],
                                start=(e == 0), stop=(e == ET - 1))
                for c in range(E // NB):
                    copy_ps(vw[:, st, c * NB:(c + 1) * NB], pss[c][:])
                nc.sync.dma_start(r128(vwloc[:])[st], vw[:, st, :])
            nc.gpsimd.collective_compute(
                "AllGather", mybir.AluOpType.bypass, replica_groups=PAIRS,
                ins=[vwloc.opt()], outs=[vwglob.opt()],
            )
            for hh in range(2):
                vg = r128(vwglob[hh])
                for st in range(QT):
                    nc.sync.dma_start(vw[:, hh * QT + st, :], vg[st])

        # ---- Phases 2-4: scores+exp, sums, attention-weighted output ----
        with (
            tc.tile_pool(name="p2c", bufs=4) as p2c,
            tc.tile_pool(name="ps_sc", bufs=sc_bufs, space="PSUM") as ps_sc,
            tc.tile_pool(name="ps_tp", bufs=1, space="PSUM") as ps_tp,
        ):
            # Scores^T tiles [j, i] via DoubleRow: lhsT = XF pair-block,
            # rhs = MT pair-rows; exp(s) - mu lands in px (fp8).
            for jt in range(ST):
                pss = [ps_sc.tile([P, NB], F32, name="sc", tag="sc")
                       for _ in range(QC)]
                for ed in range(ED):
                    lhsT = xf_s[:, 2 * ed:2 * ed + 2, jt * P:(jt + 1) * P]
                    for c in range(QC):
                        if fp8:
                            nc.tensor.matmul(
                                pss[c][:], lhsT,
                                mt[:, 2 * ed:2 * ed + 2, c * NB:(c + 1) * NB],
                                start=(ed == 0), stop=(ed == ED - 1),
                                perf_mode=DR)
                        else:
                            for k in range(2):
                                nc.tensor.matmul(
                                    pss[c][:],
                                    xf_s[:, 2 * ed + k, jt * P:(jt + 1) * P],
                                    mt[:, 2 * ed + k, c * NB:(c + 1) * NB],
                                    start=(ed == 0 and k == 0),
                                    stop=(ed == ED - 1 and k == 1))
                for c in range(QC):
                    pxb = p2c.tile([P, NB], BF16, name="pxb", tag="pxb")
                    nc.scalar.activation(pxb[:], pss[c][:], AF.Exp,
                                         scale=1.0 / (SW * 32.0),
                                         bias=ktb_s[:, jt:jt + 1])
                    nc.vector.tensor_scalar_sub(
                        px[:, jt, c * NB:(c + 1) * NB], pxb[:], MU)

            # sums[i] = 1^T PX' (partition reduce).
            if sums_offload:
                # DVE accumulation chain over the 16 j-tiles, then a GpSimd
                # cross-partition all-reduce: zero TensorE cycles.
                ADD, MUL = mybir.AluOpType.add, mybir.AluOpType.mult
                for c in range(QC):
                    acc = p2c.tile([P, NB], F32, name="sacc", tag="sacc")
                    red = p2c.tile([P, NB], F32, name="sred", tag="sred")
                    nc.vector.scalar_tensor_tensor(
                        acc[:], px[:, 0, c * NB:(c + 1) * NB], 1.0,
                        px[:, 1, c * NB:(c + 1) * NB], MUL, ADD)
                    for T in range(2, ST):
                        nc.vector.scalar_tensor_tensor(
                            acc[:], acc[:], 1.0,
                            px[:, T, c * NB:(c + 1) * NB], MUL, ADD)
                    nc.gpsimd.partition_all_reduce(
                        red[:], acc[:], channels=P,
                        reduce_op=bass_isa.ReduceOp.add)
                    nc.vector.tensor_copy(sums_sb[:, c * NB:(c + 1) * NB],
                                          red[0:1, :])
            else:
                for c in range(QC):
                    ps = ps_tp.tile([1, NB], F32, name="pssum", tag="pssum")
                    for T in range(SD):
                        if fp8:
                            nc.tensor.matmul(
                                ps[:], ones[:, :, 0:1],
                                px[:, 2 * T:2 * T + 2, c * NB:(c + 1) * NB],
                                start=(T == 0), stop=(T == SD - 1),
                                perf_mode=DR)
                        else:
                            for k in range(2):
                                nc.tensor.matmul(
                                    ps[:], ones[:, k, 0:1],
                                    px[:, 2 * T + k, c * NB:(c + 1) * NB],
                                    start=(T == 0 and k == 0),
                                    stop=(T == SD - 1 and k == 1))
                    nc.vector.tensor_copy(sums_sb[:, c * NB:(c + 1) * NB],
                                          ps[:])
            nc.sync.dma_start(sums_d, sums_sb[:])

            # OT[f, i] = VW^T PX' -> bf16 -> DRAM (normalization on host).
            for ft in range(ET):
                pss = [ps_sc.tile([P, NB], F32, name="sc", tag="sc")
                       for _ in range(QC)]
                for T in range(SD):
                    lhsT = vw[:, 2 * T:2 * T + 2, ft * P:(ft + 1) * P]
                    for c in range(QC):
                        if fp8:
                            nc.tensor.matmul(
                                pss[c][:], lhsT,
                                px[:, 2 * T:2 * T + 2, c * NB:(c + 1) * NB],
                                start=(T == 0), stop=(T == SD - 1),
                                perf_mode=DR)
                        else:
                            for k in range(2):
                                nc.tensor.matmul(
                                    pss[c][:],
                                    vw[:, 2 * T + k, ft * P:(ft + 1) * P],
                                    px[:, 2 * T + k, c * NB:(c + 1) * NB],
                                    start=(T == 0 and k == 0),
                                    stop=(T == SD - 1 and k == 1))
                for c in range(QC):
                    ot_t = p2c.tile([P, NB], BF16, name="ot", tag="ot")
                    copy_ps(ot_t[:], pss[c][:])
                    nc.sync.dma_start(
                        out_d[ft * P:(ft + 1) * P, c * NB:(c + 1) * NB],
                        ot_t[:])


def build_program(num_devices=NCORES, repeats=1, **emit_kw):
    nc = bacc.Bacc("TRN2", target_bir_lowering=False, debug=False,
                   num_devices=num_devices)
    fp8 = emit_kw.get("fp8", FP8)
    vw_fp8 = emit_kw.get("vw_fp8", VW_FP8) and fp8
    mt_fp8 = emit_kw.get("mt_fp8", MT_FP8) and fp8
    XDT = F8 if fp8 else BF16
    aps = (
        nc.dram_tensor("xh", [E, SQ], BF16, kind="ExternalInput").ap(),
        nc.dram_tensor("xh8", [E, SQ], F8 if fp8 else BF16,
                       kind="ExternalInput").ap(),
        nc.dram_tensor("xf", [E, S], XDT, kind="ExternalInput").ap(),
        nc.dram_tensor("w12", [E, E], F8 if mt_fp8 else BF16,
                       kind="ExternalInput").ap(),
        nc.dram_tensor("w34", [E, E], F8 if vw_fp8 else BF16,
                       kind="ExternalInput").ap(),
        nc.dram_tensor("ktb", [P, ST], F32, kind="ExternalInput").ap(),
        nc.dram_tensor("out", [E, SQ], BF16, kind="ExternalOutput").ap(),
        nc.dram_tensor("sums", [1, SQ], F32, kind="ExternalOutput").ap(),
    )
    with tile.TileContext(nc) as tc:
        for _ in range(repeats):
            emit_attention(tc, aps, **emit_kw)
    nc.compile()
    return nc


def host_prep(x, W1, b1, W2, b2, W3, b3, W4, b4, fp8=None, vw_fp8=None,
              mt_fp8=None):
    if fp8 is None: fp8 = FP8
    if vw_fp8 is None: vw_fp8 = VW_FP8
    if mt_fp8 is None: mt_fp8 = MT_FP8
    """Fold weights / biases; build per-core input dicts + postproc consts."""
    f32 = np.float32
    vw_fp8 = vw_fp8 and fp8
    mt_fp8 = mt_fp8 and fp8
    W12 = (W1.astype(f32) @ W2.T.astype(f32)) * f32(SW)
    W34 = (W3.astype(f32) @ W4.astype(f32)) * f32(SW)
    b4p = (b3.astype(np.float64) @ W4.astype(np.float64) + b4).astype(f32)
    w21 = (W2.astype(f32) @ b1.astype(f32))
    c21 = float(b2.astype(np.float64) @ b1.astype(np.float64))
    np_x = NP_F8 if fp8 else NP_BF16
    ws = {"w12": np.ascontiguousarray(W12.astype(NP_F8 if mt_fp8 else NP_BF16)),
          "w34": np.ascontiguousarray(W34.astype(NP_F8 if vw_fp8 else NP_BF16))}
    in_maps, css = [], []
    for b in range(B):
        xb = np.asarray(x[b], f32)
        xT = np.ascontiguousarray(xb.T)
        xf = np.ascontiguousarray(xT.astype(np_x))
        ktil = (xb @ w21 + c21) / f32(32.0)
        ktb = np.ascontiguousarray(ktil.reshape(ST, P).T.astype(f32))
        css.append((xb.sum(0) @ W34).astype(f32))
        for h in range(2):
            xh = np.ascontiguousarray(xT[:, h * SQ:(h + 1) * SQ].astype(NP_BF16))
            xh8 = np.ascontiguousarray(xf[:, h * SQ:(h + 1) * SQ])
            in_maps.append({"xh": xh, "xh8": xh8, "xf": xf, "ktb": ktb, **ws})
    return in_maps, css, b4p


def make_in_maps(x, W1, b1, W2, b2, W3, b3, W4, b4):
    return host_prep(x, W1, b1, W2, b2, W3, b3, W4, b4)[0]


_PROGRAM = None


def kernel(x, W1, b1, W2, b2, W3, b3, W4, b4):
    x, W1, b1, W2, b2, W3, b3, W4, b4 = (
        np.asarray(a) for a in (x, W1, b1, W2, b2, W3, b3, W4, b4))
    global _PROGRAM
    if _PROGRAM is None:
        _PROGRAM = build_program()
    nc = _PROGRAM
    in_maps, css, b4p = host_prep(x, W1, b1, W2, b2, W3, b3, W4, b4)
    res = run_bass_kernel_spmd(nc, in_maps, core_ids=list(range(NCORES)))
    out = np.empty((B, S, E), np.float32)
    for i in range(NCORES):
        b, h = divmod(i, 2)
        ot = np.asarray(res.results[i]["out"]).astype(np.float32)  # [E, SQ]
        sums = np.asarray(res.results[i]["sums"])[0].astype(np.float32)
        dst = out[b, h * SQ:(h + 1) * SQ, :]
        np.multiply(ot.T + MU * css[b][None, :],
                    (1.0 / (SW * (sums + MU * S)))[:, None], out=dst)
        dst += b4p[None, :]
    return out
